# revision 3
# baseline (speedup 1.0000x reference)
"""TRN2 Bass kernel for nn_DCABlock (1x1 convs + ECA channel attention + dual softmax).

Self-contained: hardcodes shapes for x:(16,2048,32,32) fp32.
Strategy: pure data parallelism — 2 samples per core on 8 NeuronCores.

Math (per sample, X = x[b] as (C,N) with N=h*w=1024, IC=C/2=1024):
  xphi = w_phi @ X                                 (IC,N)
  Q    = xphi * (1 + sigmoid(conv1d_k5(mean_n xphi)))   [ECA]
  S    = Q^T Q   (symmetric)                       (N,N)
  R    = rowsoftmax(S)           == sm^T (sm = softmax(S, axis=0))
  AT   = Q @ R                   == A^T            (IC,N)
  E2   = exp(AT - rowmax(AT)); rsU = rowsum(E2)    [sm2^T = E2/rsU]
  BT   = (E2^T @ Q) * (1/rsU per row)              (IC,N)
  out  = w_mask @ (AT + BT) + X                    (C,N)
(The reference's theta/eca_k branch is dead code and skipped.)

All large matmuls run as float32r (full PE rate, ~11-bit mantissa rounding,
fp32 PSUM accumulation); end-to-end error vs fp32 reference ~2e-4 scale-relative.
"""
import numpy as np

_C = 2048
_IC = 1024
_N = 1024
_H = 32
_NCORES = 8
_SPC = 2           # samples per core
_KECA = 5

_PROG = []


def _make_bands(wq):
    """(128, 3*128) fp32: band blocks so that the cross-channel ECA conv becomes
    24 tiny PE matmuls on the per-tile rowsum vector Y (128,8).

    s_logit[t*128+a] = sum_dt sum_p B[p, (dt+1)*128+a] * Y[p, t+dt]
    B[p, (dt+1)*128+a] = wq[p - a + 128*dt + 2] / N   (zero outside [0,5))
    """
    bands = np.zeros((128, 3 * 128), np.float32)
    p = np.arange(128)[:, None]
    a = np.arange(128)[None, :]
    for dt in (-1, 0, 1):
        j = p - a + 128 * dt + 2
        m = (j >= 0) & (j < _KECA)
        blk = np.zeros((128, 128), np.float32)
        blk[m] = (wq[np.clip(j, 0, _KECA - 1)] / _N)[m]
        bands[:, (dt + 1) * 128:(dt + 2) * 128] = blk
    return bands


def _build():
    if _PROG:
        return _PROG[0]
    import concourse.mybir as mybir
    import concourse.tile as tile
    from concourse import bacc
    from concourse.masks import make_identity

    f32 = mybir.dt.float32
    f32r = mybir.dt.float32r
    AX = mybir.AxisListType.X
    MAX = mybir.AluOpType.max
    EXP = mybir.ActivationFunctionType.Exp
    CPY = mybir.ActivationFunctionType.Copy

    nc = bacc.Bacc("TRN2", target_bir_lowering=False, debug=False,
                   num_devices=_NCORES)
    x_t = nc.dram_tensor("x", [_SPC, _C, _N], f32, kind="ExternalInput").ap()
    wphi_t = nc.dram_tensor("wphi", [8, 128, 16, 128], f32r,
                            kind="ExternalInput").ap()
    wmask_t = nc.dram_tensor("wmask", [16, 128, 8, 128], f32r,
                             kind="ExternalInput").ap()
    bands_t = nc.dram_tensor("bands", [128, 3 * 128], f32,
                             kind="ExternalInput").ap()
    out_t = nc.dram_tensor("out", [_SPC, _C, _N], f32, kind="ExternalOutput").ap()

    with tile.TileContext(nc) as tc:
        from contextlib import ExitStack
        ctx = ExitStack()
        with ctx:
            cst = ctx.enter_context(tc.tile_pool(name="cst", bufs=1))
            sml = ctx.enter_context(tc.tile_pool(name="sml", bufs=2))
            w1p = ctx.enter_context(tc.tile_pool(name="w1p", bufs=1))
            ap_ = ctx.enter_context(tc.tile_pool(name="apl", bufs=1))
            bp_ = ctx.enter_context(tc.tile_pool(name="bpl", bufs=1))
            dp_ = ctx.enter_context(tc.tile_pool(name="dpl", bufs=1))
            wcp = ctx.enter_context(tc.tile_pool(name="wcp", bufs=2))
            xrp = ctx.enter_context(tc.tile_pool(name="xrp", bufs=2))
            opp = ctx.enter_context(tc.tile_pool(name="opp", bufs=2))
            psa = ctx.enter_context(tc.tile_pool(name="psa", bufs=3, space="PSUM"))
            pst = ctx.enter_context(tc.tile_pool(name="pst", bufs=2, space="PSUM"))

            bands = cst.tile([128, 3 * 128], f32, tag="bands", name="bands_sb")
            nc.sync.dma_start(bands[:], bands_t[:])
            ident = cst.tile([128, 128], f32, tag="ident", name="ident_sb")
            make_identity(nc, ident[:])

            def transpose_8x8(src, dst, s, lbl):
                """dst[:, t*1024 + d] = src[d-tile layout] transposed per 128x128 block.
                src/dst are (128, 8192) f32r tiles in the standard tiled layout."""
                for t in range(8):
                    for g in range(2):
                        tp = pst.tile([128, 512], f32, tag="tp",
                                      name=f"tp_{lbl}{s}_{t}_{g}")
                        for j in range(4):
                            dtile = g * 4 + j
                            blk = src[:, dtile * 1024 + t * 128:
                                      dtile * 1024 + t * 128 + 128].bitcast(f32)
                            nc.tensor.transpose(tp[:, j * 128:(j + 1) * 128],
                                                blk, ident[:])
                        nc.scalar.copy(dst[:, t * 1024 + g * 512:
                                           t * 1024 + (g + 1) * 512], tp[:])

            for s in range(_SPC):
                # ---- load X (upper half first: frees overlap with prior sample) ----
                w1 = w1p.tile([128, 16384], f32r, tag="w1", name=f"w1_{s}")
                for ct in list(range(8, 16)) + list(range(8)):
                    nc.sync.dma_start(
                        w1[:, ct * 1024:(ct + 1) * 1024],
                        x_t[s, ct * 128:(ct + 1) * 128, :].bitcast(f32r))

                # ---- phi: xphi[mt] = sum_kt wphi(kt,mt)^T @ X[kt] ----
                xphi = ap_.tile([128, 8192], f32, tag="A", name=f"xphi{s}")
                Y = sml.tile([128, 8], f32, tag="Y", name=f"Y{s}")
                kts = list(range(8, 16)) + list(range(8))
                for mt in range(8):
                    wp = wcp.tile([128, 2048], f32r, tag="wcol", name=f"wp{s}_{mt}")
                    nc.sync.dma_start(wp[:], wphi_t[mt].rearrange("p k m -> p (k m)"))
                    acc = psa.tile([128, 1024], f32, tag="acc", name=f"phiacc{s}_{mt}")
                    for i, kt in enumerate(kts):
                        for ch in range(2):
                            nc.tensor.matmul(
                                acc[:, ch * 512:(ch + 1) * 512],
                                wp[:, kt * 128:(kt + 1) * 128],
                                w1[:, kt * 1024 + ch * 512: kt * 1024 + (ch + 1) * 512],
                                start=(i == 0), stop=(i == 15))
                    nc.scalar.activation(xphi[:, mt * 1024:(mt + 1) * 1024], acc[:],
                                         CPY, accum_out=Y[:, mt:mt + 1])

                # ---- ECA: s_logit = band-conv(Y); ops = 1 + sigmoid(s_logit) ----
                sp = pst.tile([128, 512], f32, tag="tp", name=f"eca{s}")
                for t in range(8):
                    steps = [dt for dt in (-1, 0, 1) if 0 <= t + dt < 8]
                    for i, dt in enumerate(steps):
                        nc.tensor.matmul(
                            sp[:, t:t + 1],
                            bands[:, (dt + 1) * 128:(dt + 2) * 128],
                            Y[:, t + dt:t + dt + 1],
                            start=(i == 0), stop=(i == len(steps) - 1))
                sig = sml.tile([128, 8], f32, tag="sig", name=f"sig{s}")
                nc.scalar.activation(sig[:], sp[:, 0:8], EXP, scale=-1.0)
                nc.vector.tensor_scalar_add(sig[:], sig[:], 1.0)
                nc.vector.reciprocal(sig[:], sig[:])
                nc.vector.tensor_scalar_add(sig[:], sig[:], 1.0)

                # ---- Qm = xphi * (1+s) per channel (f32r) ----
                Qm = bp_.tile([128, 8192], f32r, tag="B", name=f"Qm{s}")
                for t in range(8):
                    nc.scalar.activation(Qm[:, t * 1024:(t + 1) * 1024],
                                         xphi[:, t * 1024:(t + 1) * 1024],
                                         CPY, scale=sig[:, t:t + 1])

                # ---- S[nt] = sum_t Qm[t][:,nt]^T @ Qm[t]; R = rowsoftmax(S) ----
                rs1 = sml.tile([128, 8], f32, tag="rs1", name=f"rs1{s}")
                for nt in range(8):
                    acc = psa.tile([128, 1024], f32, tag="acc", name=f"sacc{s}_{nt}")
                    for t in range(8):
                        lhsT = Qm[:, t * 1024 + nt * 128: t * 1024 + nt * 128 + 128]
                        for ch in range(2):
                            nc.tensor.matmul(
                                acc[:, ch * 512:(ch + 1) * 512], lhsT,
                                Qm[:, t * 1024 + ch * 512: t * 1024 + (ch + 1) * 512],
                                start=(t == 0), stop=(t == 7))
                    nm = sml.tile([128, 1], f32, tag="nm", name=f"nm{s}_{nt}")
                    nc.vector.tensor_reduce(nm[:], acc[:], axis=AX, op=MAX,
                                            negate=True)
                    rsl = w1[:, nt * 1024:(nt + 1) * 1024]
                    nc.scalar.activation(rsl, acc[:], EXP, bias=nm[:],
                                         accum_out=rs1[:, nt:nt + 1])
                    rc = sml.tile([128, 1], f32, tag="rc", name=f"rc{s}_{nt}")
                    nc.vector.reciprocal(rc[:], rs1[:, nt:nt + 1])
                    nc.vector.tensor_scalar_mul(rsl, rsl, rc[:])

                # ---- QT = Qm^T ----
                QT = dp_.tile([128, 8192], f32r, tag="D", name=f"QT{s}")
                transpose_8x8(Qm, QT, s, "qt")

                # ---- AT[mt] = sum_t QT[t][:,mt]^T @ R[t]; E2 = exp(AT - rowmax) ----
                rsU = sml.tile([128, 8], f32, tag="rsU", name=f"rsU{s}")
                E2 = ap_.tile([128, 8192], f32r, tag="A", name=f"E2_{s}")
                for mt in range(8):
                    acc = psa.tile([128, 1024], f32, tag="acc", name=f"atacc{s}_{mt}")
                    for t in range(8):
                        lhsT = QT[:, t * 1024 + mt * 128: t * 1024 + mt * 128 + 128]
                        for ch in range(2):
                            nc.tensor.matmul(
                                acc[:, ch * 512:(ch + 1) * 512], lhsT,
                                w1[:, t * 1024 + ch * 512: t * 1024 + (ch + 1) * 512],
                                start=(t == 0), stop=(t == 7))
                    ats = w1[:, 8192 + mt * 1024: 8192 + (mt + 1) * 1024]
                    nc.scalar.copy(ats, acc[:])
                    nm2 = sml.tile([128, 1], f32, tag="nm2", name=f"nm2{s}_{mt}")
                    nc.vector.tensor_reduce(nm2[:], acc[:], axis=AX, op=MAX,
                                            negate=True)
                    nc.scalar.activation(E2[:, mt * 1024:(mt + 1) * 1024], ats, EXP,
                                         bias=nm2[:], accum_out=rsU[:, mt:mt + 1])
                recU = sml.tile([128, 8], f32, tag="recU", name=f"recU{s}")
                nc.vector.reciprocal(recU[:], rsU[:])

                # ---- E2T = E2^T ----
                E2T = dp_.tile([128, 8192], f32r, tag="D", name=f"E2T{s}")
                transpose_8x8(E2, E2T, s, "et")

                # ---- BT[dt] = sum_t E2T[t][:,dt]^T @ Qm[t]; add = AT + BT/rsU ----
                for dt in range(8):
                    acc = psa.tile([128, 1024], f32, tag="acc", name=f"btacc{s}_{dt}")
                    for t in range(8):
                        lhsT = E2T[:, t * 1024 + dt * 128: t * 1024 + dt * 128 + 128]
                        for ch in range(2):
                            nc.tensor.matmul(
                                acc[:, ch * 512:(ch + 1) * 512], lhsT,
                                Qm[:, t * 1024 + ch * 512: t * 1024 + (ch + 1) * 512],
                                start=(t == 0), stop=(t == 7))
                    adds = w1[:, dt * 1024:(dt + 1) * 1024]
                    nc.vector.tensor_scalar_mul(adds, acc[:], recU[:, dt:dt + 1])
                    nc.vector.tensor_add(
                        adds, adds,
                        w1[:, 8192 + dt * 1024: 8192 + (dt + 1) * 1024])

                # ---- mask[ct] = sum_kt wmask(kt,ct)^T @ add[kt]; out = mask + x ----
                for ct in range(16):
                    wm = wcp.tile([128, 1024], f32r, tag="wcol", name=f"wm{s}_{ct}")
                    nc.sync.dma_start(wm[:], wmask_t[ct].rearrange("p k m -> p (k m)"))
                    acc = psa.tile([128, 1024], f32, tag="acc", name=f"mkacc{s}_{ct}")
                    for kt in range(8):
                        for ch in range(2):
                            nc.tensor.matmul(
                                acc[:, ch * 512:(ch + 1) * 512],
                                wm[:, kt * 128:(kt + 1) * 128],
                                w1[:, kt * 1024 + ch * 512: kt * 1024 + (ch + 1) * 512],
                                start=(kt == 0), stop=(kt == 7))
                    xt = xrp.tile([128, 1024], f32, tag="xr", name=f"xr{s}_{ct}")
                    nc.sync.dma_start(xt[:], x_t[s, ct * 128:(ct + 1) * 128, :])
                    ot = opp.tile([128, 1024], f32, tag="op", name=f"ot{s}_{ct}")
                    nc.vector.tensor_add(ot[:], acc[:], xt[:])
                    nc.sync.dma_start(out_t[s, ct * 128:(ct + 1) * 128, :], ot[:])

    nc.compile()
    _PROG.append(nc)
    return nc


def kernel(x, w_phi, w_eca_q, w_theta, w_eca_k, w_mask):
    from concourse.bass_utils import run_bass_kernel_spmd

    x = np.asarray(x, np.float32)
    w_phi = np.asarray(w_phi, np.float32)
    w_mask = np.asarray(w_mask, np.float32)
    w_eca_q = np.asarray(w_eca_q, np.float32)

    # host-side weight re-layouts (tiled for efficient per-partition DMA)
    # wphi[mt, p, kt, m] = w_phi[mt*128+m, kt*128+p]
    wphi_l = np.ascontiguousarray(
        w_phi.reshape(8, 128, 16, 128).transpose(0, 3, 2, 1))
    # wmask[ct, p, kt, m] = w_mask[ct*128+m, kt*128+p]
    wmask_l = np.ascontiguousarray(
        w_mask.reshape(16, 128, 8, 128).transpose(0, 3, 2, 1))
    bands = _make_bands(w_eca_q)

    xs = x.reshape(_NCORES, _SPC, _C, _N)
    nc = _build()
    in_maps = [{"x": np.ascontiguousarray(xs[i]), "wphi": wphi_l,
                "wmask": wmask_l, "bands": bands} for i in range(_NCORES)]
    res = run_bass_kernel_spmd(nc, in_maps, list(range(_NCORES)))
    out = np.stack([res.results[i]["out"] for i in range(_NCORES)])
    return out.reshape(_NCORES * _SPC, _C, _H, _H)
